# revision 32
# baseline (speedup 1.0000x reference)
"""GQA attention kernel v23 for Trainium2, 8 NeuronCores.

Sharding: data-parallel over batch (4) x tensor-parallel over head groups (2).
Each core handles one (batch, head-group): 8 query heads / 2 kv heads.
o_proj is row-parallel -> host sums the 2 partial outputs per batch.

v4 vs v3:
  - Host pre-arranges xT/wq/wk/wv/wo into the on-chip [p][c][m] layouts so
    every big DMA is contiguous per partition (line rate vs ~45%).
  - Attention emitted head-outer with q-tile order [0,3,1,2] per head, so
    small q-tile units' normalize tails hide under big units' PE work.
  - o_proj is a dense tail block over all q-tiles.
  - d / broadcast PSUM share one 2-buf pool slot (fits 8 banks total).
"""

import json as _json

import numpy as np

import concourse.bass as bass
import concourse.mybir as mybir
import concourse.tile as tile

# --- walrus sync-wait legalizer (same as baseline) -------------------------
_MAX_WAITS = 1
_orig_to_json_bytes = bass.Bass.to_json_bytes


def _split_waits_json(raw: bytes) -> bytes:
    m = _json.loads(raw)
    changed = False
    for fn in m.get("functions", []):
        for bb in fn.get("blocks", []):
            out = []
            for inst in bb.get("instructions", []):
                si = inst.get("sync_info")
                waits = (si or {}).get("on_wait") or []
                if len(waits) > _MAX_WAITS:
                    changed = True
                    for k, w in enumerate(waits[:-_MAX_WAITS]):
                        out.append({
                            "debug": inst.get("debug", 0),
                            "engine": inst["engine"],
                            "ins": [], "outs": [],
                            "name": f"{inst['name']}-sw{k}",
                            "opcode": "EventSemaphore",
                            "sync_info": {"on_update": [], "on_wait": [w]},
                        })
                    si["on_wait"] = waits[-_MAX_WAITS:]
                out.append(inst)
            bb["instructions"] = out
    if not changed:
        return raw
    return _json.dumps(m).encode()


def _patched_to_json_bytes(self):
    return _split_waits_json(_orig_to_json_bytes(self))


bass.Bass.to_json_bytes = _patched_to_json_bytes
# --------------------------------------------------------------------------

B, D = 4, 2048
NH, NKV, HD = 16, 4, 128
NHL, NKVL = 8, 2          # per-core q heads / kv heads
DQ = NHL * HD             # 1024
DKV = NKVL * HD           # 256
KD = D // 128             # 16 contraction chunks
TQ = 512                  # query tile width
THETA = 10000.0
SCALE = HD ** -0.5
NCORES = 8
NSUB, CSUB = 4, KD // 4   # x tile split for early DMA completion

f16 = mybir.dt.float16
f32 = mybir.dt.float32
EXP = mybir.ActivationFunctionType.Exp
LOG = mybir.ActivationFunctionType.Ln


def build_nc(T=2048):
    njq = T // TQ
    nck = T // 128
    ts = bass.ts

    nc = bass.Bass()
    # all inputs pre-arranged host-side for contiguous per-partition DMA
    xTp = nc.dram_tensor("xTp", [128, njq, KD, TQ], f16, kind="ExternalInput")
    wqp = nc.dram_tensor("wqp", [128, KD, DQ], f16, kind="ExternalInput")
    wkp = nc.dram_tensor("wkp", [128, KD, DKV], f16, kind="ExternalInput")
    wvp = nc.dram_tensor("wvp", [128, KD, DKV], f16, kind="ExternalInput")
    wop = nc.dram_tensor("wop", [128, NHL, D], f16, kind="ExternalInput")
    cosT = nc.dram_tensor("cosT", [HD, T], f16, kind="ExternalInput")
    sinT = nc.dram_tensor("sinT", [HD, T], f16, kind="ExternalInput")
    tri = nc.dram_tensor("tri", [128, 128], f16, kind="ExternalInput")
    out = nc.dram_tensor("out", [T, D], f16, kind="ExternalOutput")

    with tile.TileContext(nc) as tc:
        with tc.tile_pool(name="res", bufs=1) as res:
            QT_sb = res.tile([128, NHL, T], f16)
            KT_sb = res.tile([128, NKVL, T], f16)
            V_sb = res.tile([128, nck, DKV], f16)
            tri_sb = res.tile([128, 128], f16)
            ones_sb = res.tile([128, 1], f16)
            onesr_sb = res.tile([1, 128], f16)
            wo_sb = res.tile([128, NHL, D], f16)
            wv_sb = res.tile([128, KD, DKV], f16)
            xt3 = [res.tile([128, CSUB, TQ], f16, name=f"xt3h_{u}")
                   for u in range(NSUB)]
            nc.vector.memset(ones_sb, 1.0)
            nc.vector.memset(onesr_sb, 1.0)

            # ---------------- Phase 1: projections + RoPE ----------------
            with tc.tile_pool(name="w1", bufs=1) as w1, \
                 tc.tile_pool(name="p1x", bufs=2) as xpool, \
                 tc.tile_pool(name="p1ps", bufs=4, space="PSUM") as pspool, \
                 tc.tile_pool(name="p1t", bufs=3) as tpool:
                wk_sb = w1.tile([128, KD, DKV], f16)
                wq_sb = w1.tile([128, KD, DQ], f16)
                cos_sb = w1.tile([128, T], f16)
                sin_sb = w1.tile([128, T], f16)

                wsrc = w1.tile([128, TQ], f16)
                wwgt = w1.tile([128, 128], f16)
                nc.vector.memset(wsrc, 0.0)
                nc.vector.memset(wwgt, 0.0)
                for wi in range(28):
                    wps = pspool.tile([128, TQ], f32, tag="ps", name=f"warm{wi}")
                    nc.tensor.matmul(wps, lhsT=wwgt, rhs=wsrc,
                                     start=True, stop=True)
                for jt in range(njq):
                    if jt == njq - 1 and njq > 1:
                        xt = xt3
                    else:
                        xt = [xpool.tile([128, CSUB, TQ], f16, tag=f"xt{u}",
                                         name=f"xt{jt}_{u}")
                              for u in range(NSUB)]
                    for u in range(NSUB):
                        nc.sync.dma_start(out=xt[u],
                                          in_=xTp[:, jt, u * CSUB:(u + 1) * CSUB, :])
                        if jt == 0 and u == 0:
                            nc.sync.dma_start(out=wk_sb, in_=wkp[:, :, :])
                    if jt == 0:
                        nc.sync.dma_start(out=wv_sb, in_=wvp[:, :, :])
                        nc.sync.dma_start(out=tri_sb, in_=tri[:, :])
                        nc.sync.dma_start(out=cos_sb, in_=cosT[:, :])
                        nc.sync.dma_start(out=sin_sb, in_=sinT[:, :])
                        nc.sync.dma_start(out=wq_sb[:, :, 0:DQ // 2],
                                          in_=wqp[:, :, 0:DQ // 2])
                    if jt == 0:
                        nc.sync.dma_start(out=wq_sb[:, :, DQ // 2:DQ],
                                          in_=wqp[:, :, DQ // 2:DQ])
                    if jt == min(2, njq - 1):
                        nc.sync.dma_start(out=wo_sb, in_=wop[:, :, :])
                    # K first (unblocks nothing downstream yet but cheap), V, Q
                    for h in range(NKVL + NHL):
                        if h < NKVL:
                            w_sb, col = wk_sb, h * 128
                            dst = KT_sb[:, h, ts(jt, TQ)]
                        else:
                            qh = h - NKVL
                            w_sb, col = wq_sb, qh * 128
                            dst = QT_sb[:, qh, ts(jt, TQ)]
                        ps = pspool.tile([128, TQ], f32, tag="ps")
                        for c in range(KD):
                            nc.tensor.matmul(
                                ps, lhsT=w_sb[:, c, col:col + 128],
                                rhs=xt[c // CSUB][:, c % CSUB, :],
                                start=(c == 0), stop=(c == KD - 1))
                        # RoPE in [head_dim, T] layout; rotate-half via two
                        # small SBUF->SBUF DMAs (engines can't partition-shift)
                        qf = tpool.tile([128, TQ], f16, tag="qf")
                        nc.scalar.copy(qf, ps)
                        qs = tpool.tile([128, TQ], f16, tag="qs")
                        nc.sync.dma_start(out=qs[0:64, :], in_=qf[64:128, :])
                        nc.sync.dma_start(out=qs[64:128, :], in_=qf[0:64, :])
                        tu = tpool.tile([128, TQ], f16, tag="tu")
                        nc.vector.tensor_mul(qs, qs, sin_sb[:, ts(jt, TQ)])
                        nc.vector.tensor_mul(tu, qf, cos_sb[:, ts(jt, TQ)])
                        nc.vector.tensor_add(dst, tu, qs)
                        if h == NKVL - 1 and not (jt == njq - 1 and njq > 1):
                            # V for this jt: natural [T, dkv] layout
                            for s in range(4):
                                pv = pspool.tile([128, DKV], f32, tag="pv")
                                for c in range(KD):
                                    nc.tensor.matmul(
                                        pv,
                                        lhsT=xt[c // CSUB][:, c % CSUB,
                                                           s * 128:(s + 1) * 128],
                                        rhs=wv_sb[:, c, :],
                                        start=(c == 0), stop=(c == KD - 1))
                                nc.scalar.copy(V_sb[:, 4 * jt + s, :], pv)

            # ---------------- Phase 2: attention + interleaved o_proj ----
            with tc.tile_pool(name="p2s", bufs=2, space="PSUM") as spool, \
                 tc.tile_pool(name="p2o", bufs=3, space="PSUM") as opool, \
                 tc.tile_pool(name="p2d", bufs=1, space="PSUM") as dpool, \
                 tc.tile_pool(name="p2p", bufs=6) as ppool, \
                 tc.tile_pool(name="p2ps", bufs=4) as pspool2, \
                 tc.tile_pool(name="p2t", bufs=2) as t2pool, \
                 tc.tile_pool(name="p2bc", bufs=3) as bcpool, \
                 tc.tile_pool(name="p2ot", bufs=njq) as otpool, \
                 tc.tile_pool(name="p2dr", bufs=3, space="DRAM") as drpool, \
                 tc.tile_pool(name="p2out", bufs=3) as outpool:
                OTu = [otpool.tile([128, NHL, TQ], f16, tag="OTu",
                                   name=f"OTu{jq}") for jq in range(njq)]
                pending = []   # deferred bc tails (1-unit delay)
                oq = []        # ready o_proj emitters, 4 per (jq, s)
                vq = []        # deferred last-jt V-projection blocks

                def emit_v3(sblk):
                    jt = njq - 1
                    pv = opool.tile([128, DKV], f32, tag="o",
                                    name=f"pv3_{sblk}")
                    for c in range(KD):
                        nc.tensor.matmul(
                            pv,
                            lhsT=xt3[c // CSUB][:, c % CSUB,
                                               sblk * 128:(sblk + 1) * 128],
                            rhs=wv_sb[:, c, :],
                            start=(c == 0), stop=(c == KD - 1))
                    nc.scalar.copy(V_sb[:, 4 * jt + sblk, :], pv)

                if njq > 1:
                    for sblk in range(4):
                        vq.append(lambda sblk=sblk: emit_v3(sblk))
                osb_cur = [None]

                def emit_oproj(jq, sblk, nt):
                    if nt == 0:
                        osb_cur[0] = outpool.tile([128, D], f16, tag="osb",
                                                  name=f"osb{jq}_{sblk}")
                    osb = osb_cur[0]
                    op_ps = opool.tile([128, 512], f32, tag="o",
                                       name=f"op{jq}_{sblk}_{nt}")
                    for hc in range(NHL):
                        nc.tensor.matmul(
                            op_ps,
                            lhsT=OTu[jq][:, hc, sblk * 128:(sblk + 1) * 128],
                            rhs=wo_sb[:, hc, ts(nt, 512)],
                            start=(hc == 0), stop=(hc == NHL - 1))
                    nc.vector.tensor_copy(osb[:, ts(nt, 512)], op_ps)
                    if nt == 3:
                        row = jq * TQ + sblk * 128
                        nc.sync.dma_start(out=out[row:row + 128, :], in_=osb)

                def drain_tail():
                    if pending:
                        pending.pop(0)()

                def drain_oq():
                    if oq:
                        oq.pop(0)()
                    elif vq:
                        vq.pop(0)()

                for jq in range(njq):
                    for h in range(NHL):
                        g = h // 4
                        drain_tail()
                        o_ps = opool.tile([128, TQ], f32, tag="o")
                        psum16 = pspool2.tile([128, TQ], f16, tag="psum16")
                        qrhs = QT_sb[:, h, ts(jq, TQ)]
                        # off-diagonal chunk pairs (full width, no mask)
                        for cp in range(2 * jq):
                            c0 = 2 * cp
                            s2 = spool.tile([128, 2 * TQ], f32, tag="s")
                            nc.tensor.matmul(s2[:, 0:TQ],
                                             lhsT=KT_sb[:, g, ts(c0, 128)],
                                             rhs=qrhs, start=True, stop=True)
                            nc.tensor.matmul(s2[:, TQ:2 * TQ],
                                             lhsT=KT_sb[:, g, ts(c0 + 1, 128)],
                                             rhs=qrhs, start=True, stop=True)
                            p2 = ppool.tile([128, 2 * TQ], f16, tag="p")
                            nc.scalar.activation(p2, s2, EXP, scale=SCALE)
                            nc.tensor.matmul(o_ps,
                                             lhsT=V_sb[:, c0, g * 128:(g + 1) * 128],
                                             rhs=p2[:, 0:TQ],
                                             start=(c0 == 0), stop=False)
                            nc.tensor.matmul(o_ps,
                                             lhsT=V_sb[:, c0 + 1, g * 128:(g + 1) * 128],
                                             rhs=p2[:, TQ:2 * TQ],
                                             start=False, stop=False)
                            if c0 == 0:
                                nc.vector.tensor_copy(psum16, p2[:, 0:TQ])
                            else:
                                nc.vector.tensor_add(psum16, psum16, p2[:, 0:TQ])
                            nc.vector.tensor_add(psum16, psum16, p2[:, TQ:2 * TQ])
                            if cp == jq - 1 or (jq == njq - 1 and cp == 4):
                                drain_oq()
                        # diagonal chunks in ragged pairs: (r0,r1) and (r2,r3)
                        for rp in range(2):
                            r0 = 2 * rp
                            sd = spool.tile([128, 2 * TQ], f32, tag="s")
                            pd = ppool.tile([128, 2 * TQ], f16, tag="p")
                            width = 0
                            offs = []
                            for rr in range(2):
                                r = r0 + rr
                                q0 = 128 * r
                                nr = TQ - q0
                                # pack contiguously; each region stays in one bank
                                off = width
                                offs.append((r, q0, nr, off))
                                nc.tensor.matmul(
                                    sd[:, off:off + nr],
                                    lhsT=KT_sb[:, g, ts(4 * jq + r, 128)],
                                    rhs=QT_sb[:, h, jq * TQ + q0:(jq + 1) * TQ],
                                    start=True, stop=True)
                                width = off + nr
                            nc.scalar.activation(pd[:, 0:width], sd[:, 0:width],
                                                 EXP, scale=SCALE)
                            first_unit = (jq == 0 and rp == 0)
                            for (r, q0, nr, off) in offs:
                                nc.vector.tensor_mul(pd[:, off:off + 128],
                                                     pd[:, off:off + 128], tri_sb)
                                first = first_unit and r == r0
                                nc.tensor.matmul(
                                    o_ps[:, q0:TQ],
                                    lhsT=V_sb[:, 4 * jq + r, g * 128:(g + 1) * 128],
                                    rhs=pd[:, off:off + nr],
                                    start=first, stop=(r == 3))
                                if first:
                                    nc.vector.tensor_copy(psum16, pd[:, 0:TQ])
                                else:
                                    nc.vector.tensor_add(psum16[:, q0:TQ],
                                                         psum16[:, q0:TQ],
                                                         pd[:, off:off + nr])
                        drain_oq()
                        # denominator + unnormalized O^T; 1/d on ACT inline
                        d_ps = dpool.tile([128, TQ], f32, tag="d", name=f"d{h}_{jq}")
                        nc.tensor.matmul(d_ps[0:1, :], lhsT=ones_sb,
                                         rhs=psum16, start=True, stop=True)
                        nc.vector.tensor_copy(OTu[jq][:, h, :], o_ps)
                        lnd = t2pool.tile([1, TQ], f32, tag="lnd")
                        nc.scalar.activation(lnd, d_ps[0:1, :], LOG)
                        rden = t2pool.tile([1, TQ], f16, tag="rden",
                                           name=f"rden{h}_{jq}")
                        nc.scalar.activation(rden, lnd, EXP, scale=-1.0)

                        # broadcast 1/d across partitions. DRAM bounce
                        # (2 DMAs, high latency but off-engine) for most
                        # units; low-latency bc-matmul for the final two
                        # units, whose latency gates the last o_proj blocks.
                        bc_sb = bcpool.tile([128, TQ], f16, tag="bc",
                                            name=f"bc{h}_{jq}")
                        if jq == njq - 1 and h >= NHL - 2:
                            bc_ps = dpool.tile([128, TQ], f32, tag="d",
                                               name=f"bcp{h}_{jq}")
                            nc.tensor.matmul(bc_ps, lhsT=onesr_sb, rhs=rden,
                                             start=True, stop=True)
                            nc.vector.tensor_copy(bc_sb, bc_ps)
                        else:
                            scr = drpool.tile([1, TQ], f16, tag="dsc",
                                              name=f"scr{h}_{jq}")
                            nc.sync.dma_start(out=scr, in_=rden)
                            bsrc, _ = bass.broadcast_tensor_aps(scr[:, :],
                                                                bc_sb[:, :])
                            nc.sync.dma_start(out=bc_sb, in_=bsrc)

                        def _tail(h=h, jq=jq, bc_sb=bc_sb):
                            nc.vector.tensor_mul(OTu[jq][:, h, :],
                                                 OTu[jq][:, h, :], bc_sb)
                        pending.append(_tail)
                    # queue this jq's o_proj blocks (ready once tails drain)
                    for sblk in range(4):
                        for nt in range(4):
                            oq.append(lambda jq=jq, sblk=sblk, nt=nt:
                                      emit_oproj(jq, sblk, nt))
                # drain everything left: normalize-muls first so
                # the remaining o_proj blocks never wait on them
                while pending:
                    drain_tail()
                while oq or vq:
                    drain_oq()
    return nc


def rope_tables(T=2048):
    inv = 1.0 / (THETA ** (np.arange(0, HD, 2, dtype=np.float64) / HD))
    t = np.arange(T, dtype=np.float64)
    freqs = np.outer(t, inv)
    emb = np.concatenate([freqs, freqs], -1)      # [T, 128]
    cos = np.cos(emb).T.astype(np.float16)
    sin = np.sin(emb).T.astype(np.float64)
    sin_signed = sin.copy()
    sin_signed[:64] *= -1.0                        # rotate_half sign fold
    return (np.ascontiguousarray(cos),
            np.ascontiguousarray(sin_signed.astype(np.float16)))


def tri_mask():
    k = np.arange(128)[:, None]
    q = np.arange(128)[None, :]
    return np.ascontiguousarray((k <= q).astype(np.float16))


def prep_w(w):
    """[D, M] -> [128, D//128, M] with row index d = c*128 + p."""
    Dd, M = w.shape
    return np.ascontiguousarray(
        w.reshape(Dd // 128, 128, M).transpose(1, 0, 2))


def prep_x(xb, T):
    """x[b] [T, D] -> xT pre-arranged [128, njq, KD, TQ]."""
    njq = T // TQ
    xT = xb.T  # [D, T]
    return np.ascontiguousarray(
        xT.reshape(KD, 128, njq, TQ).transpose(1, 2, 0, 3))


def build_in_maps(x, wq, wk, wv, wo, T=2048):
    cos, sin_s = rope_tables(T)
    tri = tri_mask()
    wq16 = np.asarray(wq).astype(np.float16)
    wk16 = np.asarray(wk).astype(np.float16)
    wv16 = np.asarray(wv).astype(np.float16)
    wo16 = np.asarray(wo).astype(np.float16)
    in_maps = []
    for core in range(NCORES):
        b, hg = core // 2, core % 2
        in_maps.append({
            "xTp": prep_x(np.asarray(x)[b].astype(np.float16), T),
            "wqp": prep_w(wq16[:, hg * DQ:(hg + 1) * DQ]),
            "wkp": prep_w(wk16[:, hg * DKV:(hg + 1) * DKV]),
            "wvp": prep_w(wv16[:, hg * DKV:(hg + 1) * DKV]),
            "wop": prep_w(wo16[hg * DQ:(hg + 1) * DQ, :]),
            "cosT": cos, "sinT": sin_s, "tri": tri,
        })
    return in_maps


_NC_CACHE = {}


def get_nc(T=2048):
    if T not in _NC_CACHE:
        _NC_CACHE[T] = build_nc(T)
    return _NC_CACHE[T]


def run(inputs, trace=False, **kw):
    """Returns (full_output [B,T,D] f32, BassKernelResults)."""
    from concourse import bass_utils
    x = np.asarray(inputs["x"], dtype=np.float32)
    T = x.shape[1]
    nc = get_nc(T)
    in_maps = build_in_maps(x, inputs["wq"], inputs["wk"], inputs["wv"],
                            inputs["wo"], T)
    res = bass_utils.run_bass_kernel_spmd(nc, in_maps,
                                          core_ids=list(range(NCORES)),
                                          trace=trace, **kw)
    outs = [np.asarray(r["out"]) for r in res.results]
    full = np.empty((B, T, D), dtype=np.float32)
    for b in range(B):
        full[b] = outs[2 * b].astype(np.float32) + outs[2 * b + 1].astype(np.float32)
    return full, res


def kernel(x, mask, wq, wk, wv, wo):
    full, _ = run({"x": x, "mask": mask, "wq": wq, "wk": wk, "wv": wv, "wo": wo})
    return full


# revision 33
# speedup vs baseline: 1.2196x; 1.2196x over previous
"""GQA attention kernel v22 for Trainium2, 8 NeuronCores.

Sharding: data-parallel over batch (4) x tensor-parallel over head groups (2).
Each core handles one (batch, head-group): 8 query heads / 2 kv heads.
o_proj is row-parallel -> host sums the 2 partial outputs per batch.

v4 vs v3:
  - Host pre-arranges xT/wq/wk/wv/wo into the on-chip [p][c][m] layouts so
    every big DMA is contiguous per partition (line rate vs ~45%).
  - Attention emitted head-outer with q-tile order [0,3,1,2] per head, so
    small q-tile units' normalize tails hide under big units' PE work.
  - o_proj is a dense tail block over all q-tiles.
  - d / broadcast PSUM share one 2-buf pool slot (fits 8 banks total).
"""

import json as _json

import numpy as np

import concourse.bass as bass
import concourse.mybir as mybir
import concourse.tile as tile

# --- walrus sync-wait legalizer (same as baseline) -------------------------
_MAX_WAITS = 1
_orig_to_json_bytes = bass.Bass.to_json_bytes


def _split_waits_json(raw: bytes) -> bytes:
    m = _json.loads(raw)
    changed = False
    for fn in m.get("functions", []):
        for bb in fn.get("blocks", []):
            out = []
            for inst in bb.get("instructions", []):
                si = inst.get("sync_info")
                waits = (si or {}).get("on_wait") or []
                if len(waits) > _MAX_WAITS:
                    changed = True
                    for k, w in enumerate(waits[:-_MAX_WAITS]):
                        out.append({
                            "debug": inst.get("debug", 0),
                            "engine": inst["engine"],
                            "ins": [], "outs": [],
                            "name": f"{inst['name']}-sw{k}",
                            "opcode": "EventSemaphore",
                            "sync_info": {"on_update": [], "on_wait": [w]},
                        })
                    si["on_wait"] = waits[-_MAX_WAITS:]
                out.append(inst)
            bb["instructions"] = out
    if not changed:
        return raw
    return _json.dumps(m).encode()


def _patched_to_json_bytes(self):
    return _split_waits_json(_orig_to_json_bytes(self))


bass.Bass.to_json_bytes = _patched_to_json_bytes
# --------------------------------------------------------------------------

B, D = 4, 2048
NH, NKV, HD = 16, 4, 128
NHL, NKVL = 8, 2          # per-core q heads / kv heads
DQ = NHL * HD             # 1024
DKV = NKVL * HD           # 256
KD = D // 128             # 16 contraction chunks
TQ = 512                  # query tile width
THETA = 10000.0
SCALE = HD ** -0.5
NCORES = 8
NSUB, CSUB = 4, KD // 4   # x tile split for early DMA completion

f16 = mybir.dt.float16
f32 = mybir.dt.float32
EXP = mybir.ActivationFunctionType.Exp
LOG = mybir.ActivationFunctionType.Ln


def build_nc(T=2048):
    njq = T // TQ
    nck = T // 128
    ts = bass.ts

    nc = bass.Bass()
    # all inputs pre-arranged host-side for contiguous per-partition DMA
    xTp = nc.dram_tensor("xTp", [128, njq, KD, TQ], f16, kind="ExternalInput")
    wqp = nc.dram_tensor("wqp", [128, KD, DQ], f16, kind="ExternalInput")
    wkp = nc.dram_tensor("wkp", [128, KD, DKV], f16, kind="ExternalInput")
    wvp = nc.dram_tensor("wvp", [128, KD, DKV], f16, kind="ExternalInput")
    wop = nc.dram_tensor("wop", [128, NHL, D], f16, kind="ExternalInput")
    cosT = nc.dram_tensor("cosT", [HD, T], f16, kind="ExternalInput")
    sinT = nc.dram_tensor("sinT", [HD, T], f16, kind="ExternalInput")
    tri = nc.dram_tensor("tri", [128, 128], f16, kind="ExternalInput")
    out = nc.dram_tensor("out", [T, D], f16, kind="ExternalOutput")

    with tile.TileContext(nc) as tc:
        with tc.tile_pool(name="res", bufs=1) as res:
            QT_sb = res.tile([128, NHL, T], f16)
            KT_sb = res.tile([128, NKVL, T], f16)
            V_sb = res.tile([128, nck, DKV], f16)
            tri_sb = res.tile([128, 128], f16)
            ones_sb = res.tile([128, 1], f16)
            onesr_sb = res.tile([1, 128], f16)
            wo_sb = res.tile([128, NHL, D], f16)
            wv_sb = res.tile([128, KD, DKV], f16)
            xt3 = [res.tile([128, CSUB, TQ], f16, name=f"xt3h_{u}")
                   for u in range(NSUB)]
            nc.vector.memset(ones_sb, 1.0)
            nc.vector.memset(onesr_sb, 1.0)

            # ---------------- Phase 1: projections + RoPE ----------------
            with tc.tile_pool(name="w1", bufs=1) as w1, \
                 tc.tile_pool(name="p1x", bufs=2) as xpool, \
                 tc.tile_pool(name="p1ps", bufs=4, space="PSUM") as pspool, \
                 tc.tile_pool(name="p1t", bufs=3) as tpool:
                wk_sb = w1.tile([128, KD, DKV], f16)
                wq_sb = w1.tile([128, KD, DQ], f16)
                cos_sb = w1.tile([128, T], f16)
                sin_sb = w1.tile([128, T], f16)

                wsrc = w1.tile([128, TQ], f16)
                wwgt = w1.tile([128, 128], f16)
                nc.vector.memset(wsrc, 0.0)
                nc.vector.memset(wwgt, 0.0)
                for wi in range(28):
                    wps = pspool.tile([128, TQ], f32, tag="ps", name=f"warm{wi}")
                    nc.tensor.matmul(wps, lhsT=wwgt, rhs=wsrc,
                                     start=True, stop=True)
                for jt in range(njq):
                    if jt == njq - 1 and njq > 1:
                        xt = xt3
                    else:
                        xt = [xpool.tile([128, CSUB, TQ], f16, tag=f"xt{u}",
                                         name=f"xt{jt}_{u}")
                              for u in range(NSUB)]
                    for u in range(NSUB):
                        nc.sync.dma_start(out=xt[u],
                                          in_=xTp[:, jt, u * CSUB:(u + 1) * CSUB, :])
                        if jt == 0 and u == 0:
                            nc.sync.dma_start(out=wk_sb, in_=wkp[:, :, :])
                    if jt == 0:
                        nc.sync.dma_start(out=wv_sb, in_=wvp[:, :, :])
                        nc.sync.dma_start(out=tri_sb, in_=tri[:, :])
                        nc.sync.dma_start(out=cos_sb, in_=cosT[:, :])
                        nc.sync.dma_start(out=sin_sb, in_=sinT[:, :])
                        nc.sync.dma_start(out=wq_sb[:, :, 0:DQ // 2],
                                          in_=wqp[:, :, 0:DQ // 2])
                    if jt == 0:
                        nc.sync.dma_start(out=wq_sb[:, :, DQ // 2:DQ],
                                          in_=wqp[:, :, DQ // 2:DQ])
                    if jt == min(2, njq - 1):
                        nc.sync.dma_start(out=wo_sb, in_=wop[:, :, :])
                    # K first (unblocks nothing downstream yet but cheap), V, Q
                    for h in range(NKVL + NHL):
                        if h < NKVL:
                            w_sb, col = wk_sb, h * 128
                            dst = KT_sb[:, h, ts(jt, TQ)]
                        else:
                            qh = h - NKVL
                            w_sb, col = wq_sb, qh * 128
                            dst = QT_sb[:, qh, ts(jt, TQ)]
                        ps = pspool.tile([128, TQ], f32, tag="ps")
                        for c in range(KD):
                            nc.tensor.matmul(
                                ps, lhsT=w_sb[:, c, col:col + 128],
                                rhs=xt[c // CSUB][:, c % CSUB, :],
                                start=(c == 0), stop=(c == KD - 1))
                        # RoPE in [head_dim, T] layout; rotate-half via two
                        # small SBUF->SBUF DMAs (engines can't partition-shift)
                        qf = tpool.tile([128, TQ], f16, tag="qf")
                        nc.scalar.copy(qf, ps)
                        qs = tpool.tile([128, TQ], f16, tag="qs")
                        nc.sync.dma_start(out=qs[0:64, :], in_=qf[64:128, :])
                        nc.sync.dma_start(out=qs[64:128, :], in_=qf[0:64, :])
                        tu = tpool.tile([128, TQ], f16, tag="tu")
                        nc.vector.tensor_mul(qs, qs, sin_sb[:, ts(jt, TQ)])
                        nc.vector.tensor_mul(tu, qf, cos_sb[:, ts(jt, TQ)])
                        nc.vector.tensor_add(dst, tu, qs)
                        if h == NKVL - 1 and not (jt == njq - 1 and njq > 1):
                            # V for this jt: natural [T, dkv] layout
                            for s in range(4):
                                pv = pspool.tile([128, DKV], f32, tag="pv")
                                for c in range(KD):
                                    nc.tensor.matmul(
                                        pv,
                                        lhsT=xt[c // CSUB][:, c % CSUB,
                                                           s * 128:(s + 1) * 128],
                                        rhs=wv_sb[:, c, :],
                                        start=(c == 0), stop=(c == KD - 1))
                                nc.scalar.copy(V_sb[:, 4 * jt + s, :], pv)

            # ---------------- Phase 2: attention + interleaved o_proj ----
            with tc.tile_pool(name="p2s", bufs=2, space="PSUM") as spool, \
                 tc.tile_pool(name="p2o", bufs=3, space="PSUM") as opool, \
                 tc.tile_pool(name="p2d", bufs=1, space="PSUM") as dpool, \
                 tc.tile_pool(name="p2p", bufs=5) as ppool, \
                 tc.tile_pool(name="p2ps", bufs=4) as pspool2, \
                 tc.tile_pool(name="p2t", bufs=2) as t2pool, \
                 tc.tile_pool(name="p2bc", bufs=3) as bcpool, \
                 tc.tile_pool(name="p2ot", bufs=njq) as otpool, \
                 tc.tile_pool(name="p2dr", bufs=3, space="DRAM") as drpool, \
                 tc.tile_pool(name="p2out", bufs=3) as outpool:
                OTu = [otpool.tile([128, NHL, TQ], f16, tag="OTu",
                                   name=f"OTu{jq}") for jq in range(njq)]
                pending = []   # deferred bc tails (1-unit delay)
                oq = []        # ready o_proj emitters, 4 per (jq, s)
                vq = []        # deferred last-jt V-projection blocks

                def emit_v3(sblk):
                    jt = njq - 1
                    pv = opool.tile([128, DKV], f32, tag="o",
                                    name=f"pv3_{sblk}")
                    for c in range(KD):
                        nc.tensor.matmul(
                            pv,
                            lhsT=xt3[c // CSUB][:, c % CSUB,
                                               sblk * 128:(sblk + 1) * 128],
                            rhs=wv_sb[:, c, :],
                            start=(c == 0), stop=(c == KD - 1))
                    nc.scalar.copy(V_sb[:, 4 * jt + sblk, :], pv)

                if njq > 1:
                    for sblk in range(4):
                        vq.append(lambda sblk=sblk: emit_v3(sblk))
                osb_cur = [None]

                def emit_oproj(jq, sblk, nt):
                    if nt == 0:
                        osb_cur[0] = outpool.tile([128, D], f16, tag="osb",
                                                  name=f"osb{jq}_{sblk}")
                    osb = osb_cur[0]
                    op_ps = opool.tile([128, 512], f32, tag="o",
                                       name=f"op{jq}_{sblk}_{nt}")
                    for hc in range(NHL):
                        nc.tensor.matmul(
                            op_ps,
                            lhsT=OTu[jq][:, hc, sblk * 128:(sblk + 1) * 128],
                            rhs=wo_sb[:, hc, ts(nt, 512)],
                            start=(hc == 0), stop=(hc == NHL - 1))
                    nc.vector.tensor_copy(osb[:, ts(nt, 512)], op_ps)
                    if nt == 3:
                        row = jq * TQ + sblk * 128
                        nc.sync.dma_start(out=out[row:row + 128, :], in_=osb)

                def drain_tail():
                    if pending:
                        pending.pop(0)()

                def drain_oq():
                    if oq:
                        oq.pop(0)()
                    elif vq:
                        vq.pop(0)()

                for jq in range(njq):
                    for h in range(NHL):
                        g = h // 4
                        drain_tail()
                        o_ps = opool.tile([128, TQ], f32, tag="o")
                        psum16 = pspool2.tile([128, TQ], f16, tag="psum16")
                        qrhs = QT_sb[:, h, ts(jq, TQ)]
                        # off-diagonal chunk pairs (full width, no mask)
                        for cp in range(2 * jq):
                            c0 = 2 * cp
                            s2 = spool.tile([128, 2 * TQ], f32, tag="s")
                            nc.tensor.matmul(s2[:, 0:TQ],
                                             lhsT=KT_sb[:, g, ts(c0, 128)],
                                             rhs=qrhs, start=True, stop=True)
                            nc.tensor.matmul(s2[:, TQ:2 * TQ],
                                             lhsT=KT_sb[:, g, ts(c0 + 1, 128)],
                                             rhs=qrhs, start=True, stop=True)
                            p2 = ppool.tile([128, 2 * TQ], f16, tag="p")
                            nc.scalar.activation(p2, s2, EXP, scale=SCALE)
                            nc.tensor.matmul(o_ps,
                                             lhsT=V_sb[:, c0, g * 128:(g + 1) * 128],
                                             rhs=p2[:, 0:TQ],
                                             start=(c0 == 0), stop=False)
                            nc.tensor.matmul(o_ps,
                                             lhsT=V_sb[:, c0 + 1, g * 128:(g + 1) * 128],
                                             rhs=p2[:, TQ:2 * TQ],
                                             start=False, stop=False)
                            if c0 == 0:
                                nc.vector.tensor_copy(psum16, p2[:, 0:TQ])
                            else:
                                nc.vector.tensor_add(psum16, psum16, p2[:, 0:TQ])
                            nc.vector.tensor_add(psum16, psum16, p2[:, TQ:2 * TQ])
                            if cp == jq - 1:
                                drain_oq()
                        # diagonal chunks in ragged pairs: (r0,r1) and (r2,r3)
                        for rp in range(2):
                            r0 = 2 * rp
                            sd = spool.tile([128, 2 * TQ], f32, tag="s")
                            pd = ppool.tile([128, 2 * TQ], f16, tag="p")
                            width = 0
                            offs = []
                            for rr in range(2):
                                r = r0 + rr
                                q0 = 128 * r
                                nr = TQ - q0
                                # pack contiguously; each region stays in one bank
                                off = width
                                offs.append((r, q0, nr, off))
                                nc.tensor.matmul(
                                    sd[:, off:off + nr],
                                    lhsT=KT_sb[:, g, ts(4 * jq + r, 128)],
                                    rhs=QT_sb[:, h, jq * TQ + q0:(jq + 1) * TQ],
                                    start=True, stop=True)
                                width = off + nr
                            nc.scalar.activation(pd[:, 0:width], sd[:, 0:width],
                                                 EXP, scale=SCALE)
                            first_unit = (jq == 0 and rp == 0)
                            for (r, q0, nr, off) in offs:
                                nc.vector.tensor_mul(pd[:, off:off + 128],
                                                     pd[:, off:off + 128], tri_sb)
                                first = first_unit and r == r0
                                nc.tensor.matmul(
                                    o_ps[:, q0:TQ],
                                    lhsT=V_sb[:, 4 * jq + r, g * 128:(g + 1) * 128],
                                    rhs=pd[:, off:off + nr],
                                    start=first, stop=(r == 3))
                                if first:
                                    nc.vector.tensor_copy(psum16, pd[:, 0:TQ])
                                else:
                                    nc.vector.tensor_add(psum16[:, q0:TQ],
                                                         psum16[:, q0:TQ],
                                                         pd[:, off:off + nr])
                        drain_oq()
                        # denominator + unnormalized O^T; 1/d on ACT inline
                        d_ps = dpool.tile([128, TQ], f32, tag="d", name=f"d{h}_{jq}")
                        nc.tensor.matmul(d_ps[0:1, :], lhsT=ones_sb,
                                         rhs=psum16, start=True, stop=True)
                        nc.vector.tensor_copy(OTu[jq][:, h, :], o_ps)
                        lnd = t2pool.tile([1, TQ], f32, tag="lnd")
                        nc.scalar.activation(lnd, d_ps[0:1, :], LOG)
                        rden = t2pool.tile([1, TQ], f16, tag="rden",
                                           name=f"rden{h}_{jq}")
                        nc.scalar.activation(rden, lnd, EXP, scale=-1.0)

                        # broadcast 1/d across partitions. DRAM bounce
                        # (2 DMAs, high latency but off-engine) for most
                        # units; low-latency bc-matmul for the final two
                        # units, whose latency gates the last o_proj blocks.
                        bc_sb = bcpool.tile([128, TQ], f16, tag="bc",
                                            name=f"bc{h}_{jq}")
                        if jq == njq - 1 and h >= NHL - 2:
                            bc_ps = dpool.tile([128, TQ], f32, tag="d",
                                               name=f"bcp{h}_{jq}")
                            nc.tensor.matmul(bc_ps, lhsT=onesr_sb, rhs=rden,
                                             start=True, stop=True)
                            nc.vector.tensor_copy(bc_sb, bc_ps)
                        else:
                            scr = drpool.tile([1, TQ], f16, tag="dsc",
                                              name=f"scr{h}_{jq}")
                            nc.sync.dma_start(out=scr, in_=rden)
                            bsrc, _ = bass.broadcast_tensor_aps(scr[:, :],
                                                                bc_sb[:, :])
                            nc.sync.dma_start(out=bc_sb, in_=bsrc)

                        def _tail(h=h, jq=jq, bc_sb=bc_sb):
                            nc.vector.tensor_mul(OTu[jq][:, h, :],
                                                 OTu[jq][:, h, :], bc_sb)
                        pending.append(_tail)
                    # queue this jq's o_proj blocks (ready once tails drain)
                    for sblk in range(4):
                        for nt in range(4):
                            oq.append(lambda jq=jq, sblk=sblk, nt=nt:
                                      emit_oproj(jq, sblk, nt))
                # drain everything left: normalize-muls first so
                # the remaining o_proj blocks never wait on them
                while pending:
                    drain_tail()
                while oq or vq:
                    drain_oq()
    return nc


def rope_tables(T=2048):
    inv = 1.0 / (THETA ** (np.arange(0, HD, 2, dtype=np.float64) / HD))
    t = np.arange(T, dtype=np.float64)
    freqs = np.outer(t, inv)
    emb = np.concatenate([freqs, freqs], -1)      # [T, 128]
    cos = np.cos(emb).T.astype(np.float16)
    sin = np.sin(emb).T.astype(np.float64)
    sin_signed = sin.copy()
    sin_signed[:64] *= -1.0                        # rotate_half sign fold
    return (np.ascontiguousarray(cos),
            np.ascontiguousarray(sin_signed.astype(np.float16)))


def tri_mask():
    k = np.arange(128)[:, None]
    q = np.arange(128)[None, :]
    return np.ascontiguousarray((k <= q).astype(np.float16))


def prep_w(w):
    """[D, M] -> [128, D//128, M] with row index d = c*128 + p."""
    Dd, M = w.shape
    return np.ascontiguousarray(
        w.reshape(Dd // 128, 128, M).transpose(1, 0, 2))


def prep_x(xb, T):
    """x[b] [T, D] -> xT pre-arranged [128, njq, KD, TQ]."""
    njq = T // TQ
    xT = xb.T  # [D, T]
    return np.ascontiguousarray(
        xT.reshape(KD, 128, njq, TQ).transpose(1, 2, 0, 3))


def build_in_maps(x, wq, wk, wv, wo, T=2048):
    cos, sin_s = rope_tables(T)
    tri = tri_mask()
    wq16 = np.asarray(wq).astype(np.float16)
    wk16 = np.asarray(wk).astype(np.float16)
    wv16 = np.asarray(wv).astype(np.float16)
    wo16 = np.asarray(wo).astype(np.float16)
    in_maps = []
    for core in range(NCORES):
        b, hg = core // 2, core % 2
        in_maps.append({
            "xTp": prep_x(np.asarray(x)[b].astype(np.float16), T),
            "wqp": prep_w(wq16[:, hg * DQ:(hg + 1) * DQ]),
            "wkp": prep_w(wk16[:, hg * DKV:(hg + 1) * DKV]),
            "wvp": prep_w(wv16[:, hg * DKV:(hg + 1) * DKV]),
            "wop": prep_w(wo16[hg * DQ:(hg + 1) * DQ, :]),
            "cosT": cos, "sinT": sin_s, "tri": tri,
        })
    return in_maps


_NC_CACHE = {}


def get_nc(T=2048):
    if T not in _NC_CACHE:
        _NC_CACHE[T] = build_nc(T)
    return _NC_CACHE[T]


def run(inputs, trace=False, **kw):
    """Returns (full_output [B,T,D] f32, BassKernelResults)."""
    from concourse import bass_utils
    x = np.asarray(inputs["x"], dtype=np.float32)
    T = x.shape[1]
    nc = get_nc(T)
    in_maps = build_in_maps(x, inputs["wq"], inputs["wk"], inputs["wv"],
                            inputs["wo"], T)
    res = bass_utils.run_bass_kernel_spmd(nc, in_maps,
                                          core_ids=list(range(NCORES)),
                                          trace=trace, **kw)
    outs = [np.asarray(r["out"]) for r in res.results]
    full = np.empty((B, T, D), dtype=np.float32)
    for b in range(B):
        full[b] = outs[2 * b].astype(np.float32) + outs[2 * b + 1].astype(np.float32)
    return full, res


def kernel(x, mask, wq, wk, wv, wo):
    full, _ = run({"x": x, "mask": mask, "wq": wq, "wk": wk, "wv": wv, "wo": wo})
    return full
